# revision 42
# baseline (speedup 1.0000x reference)
"""Causal multi-head attention (B=128, T=256, C=384, H=6, Dh=64) on 8 TRN2
NeuronCores, data-parallel over batch (16 batches per core, no collectives).

Layout strategy per core (v7 — transposed scores, fused denominator,
depth-3 attention software pipeline):
  - host pre-transposes x to xT [b, C, T] and casts activations/weights to bf16
  - QT/KT computed as [D, T] (Dh on partitions); V computed as [T, H, 65+pad]
    with a constant-1.0 65th column per head ("v_aug")
  - scores are computed TRANSPOSED: S_T[ts, tq] with kt as the stationary
    operand; block layout per sub [tri(ts0,tq0) | full(ts0,tq1) |
    tri(ts1,tq1)] so the ts0 row is ONE N=256 matmul; emission alternates
    subs (PE row-groups 0-63 / 64-127) for tile-level overlap
  - exp on ACT (one strided call per pair covering both subs); causal mask
    via GpSimd affine_select (idle engine) in steady state, via DVE multiply
    in the last group where chain latency is exposed
  - AV: lhsT = P_T block, rhs = v_aug -> O lands [tq, d] in PSUM and the
    softmax denominator appears for free in column 64 of each 65-wide block
    (66-stride blocks keep PSUM outputs 8B-aligned)
  - normalize = DVE reciprocal [128,4] + ONE tensor_tensor multiply with the
    reciprocal broadcast along d (per-partition = per-tq -> cheap), which also
    serves as the PSUM->SBUF evacuation (bf16 cast)
  - per batch, O [tq, 384] is DMA-XBAR-transposed (2 calls) to OT [d, tq] for
    the output projection (32 transposes total vs 96 for per-pair P^T)
  - x loads ride the GpSimd SWDGE queue; weights split across sync/scalar/
    gpsimd queues; y stores on sync (keeps the SWDGE drain off the tail)
  - attention runs as a depth-3 software pipeline (AV+norm of att k emitted
    after scores/exp/mask of att k+2) with group g+1's QK/V projection
    matmuls as PE filler; y-projections are split into half-batch units,
    deferred behind a small reserve so the last group (which has no
    next-group chunks) still has PE filler work
"""

import sys

sys.path.insert(0, "/opt/trn_rl_repo")

import numpy as np
import ml_dtypes

import concourse.bass as bass
import concourse.tile as tile
from concourse import mybir
from concourse.bass_utils import run_bass_kernel_spmd
from concourse.masks import make_identity

def split_multi_waits(nc):
    """This walrus build accepts at most one sync-wait command per
    instruction; hoist extra waits into standalone InstEventSemaphore
    instructions on the same engine queue (queue waits run in order before
    the original instruction, so semantics are preserved)."""
    ctr = [0]

    def mk(engine, wait):
        ctr[0] += 1
        return mybir.InstEventSemaphore(
            name=f"WSPLIT-{ctr[0]}",
            engine=engine,
            ins=[],
            outs=[],
            sync_info=mybir.SyncInfo(on_wait=[wait], on_update=[]),
        )

    for f in nc.m.functions:
        for blk in f.blocks:
            insts = blk.instructions
            out = []
            for inst in insts:
                si = inst.sync_info
                if si is not None and len(si.on_wait) > 1:
                    waits = list(si.on_wait)
                    for w in waits[:-1]:
                        out.append(mk(inst.engine, w))
                    inst.sync_info = mybir.SyncInfo(
                        on_wait=[waits[-1]], on_update=list(si.on_update)
                    )
                out.append(inst)
            insts[:] = out
    return nc


N_CORES = 8
B, T, C = 128, 256, 384
H, DH = 6, 64
BL = B // N_CORES  # batches per core
GB = 2  # batches per projection group (N = GB*T = 512 <= one PSUM bank fp32)
NG = BL // GB
BF16 = mybir.dt.bfloat16
FP32 = mybir.dt.float32
AFT = mybir.ActivationFunctionType
SCALE = DH**-0.5  # 0.125


def build_kernel() -> bass.Bass:
    nc = bass.Bass()
    xT = nc.dram_tensor("xT", [BL, C, T], BF16, kind="ExternalInput")
    wqt = nc.dram_tensor("wqt", [C, C], BF16, kind="ExternalInput")  # Wq.T [C, D]
    wkt = nc.dram_tensor("wkt", [C, C], BF16, kind="ExternalInput")
    wvt = nc.dram_tensor("wvt", [C, C], BF16, kind="ExternalInput")
    wot = nc.dram_tensor("wot", [C, C], BF16, kind="ExternalInput")  # Wo.T [D, C]
    y = nc.dram_tensor("y", [BL, T, C], BF16, kind="ExternalOutput")

    with tile.TileContext(nc) as tc:
        with (
            tc.tile_pool(name="const", bufs=1) as const,
            tc.tile_pool(name="xp", bufs=2) as xp,
            tc.tile_pool(name="qkv", bufs=2) as qkv,
            tc.tile_pool(name="pp", bufs=6) as pp,
            tc.tile_pool(name="st", bufs=6) as st,
            tc.tile_pool(name="osb", bufs=4) as osb,
            tc.tile_pool(name="otp", bufs=6) as otp,
            tc.tile_pool(name="yp", bufs=6) as yp,
            tc.tile_pool(name="psProj", bufs=3, space="PSUM") as psProj,
            tc.tile_pool(name="psSc", bufs=3, space="PSUM") as psSc,
            tc.tile_pool(name="psPo", bufs=2, space="PSUM") as psPo,
        ):
            # prefetch x for group 0 (SWDGE queue) ahead of the (larger)
            # weight DMAs (sync queue) so the first projections start ASAP
            xt0 = xp.tile([128, 3, GB, T], BF16, name="xt_g0")
            for bi in range(GB):
                nc.gpsimd.dma_start(
                    out=xt0[:, :, bi, :],
                    in_=xT[bi].rearrange("(k p) t -> p k t", p=128),
                )
            # multiplicative causal mask (used by the DVE tail-mask path)
            mtriT = const.tile([128, 128], BF16)
            nc.gpsimd.memset(mtriT, 1.0)
            nc.gpsimd.affine_select(
                out=mtriT, in_=mtriT,
                compare_op=mybir.AluOpType.is_ge,
                fill=0.0, base=0, pattern=[[1, 128]], channel_multiplier=-1,
            )
            seed = mtriT  # reuse as the dummy-exp input
            # identity for the final batch's PE-mode O-transpose
            ident = const.tile([128, 128], BF16)
            make_identity(nc, ident)
            dummy = const.tile([128, 2], FP32)
            nc.scalar.activation(dummy, seed[:, 0:2], AFT.Exp, scale=1.0)

            # weight loads spread over the three DMA-capable queues, ordered
            # by first use, so the first projections start ~1us in
            w_sb = {}
            for name, dram, eng in (
                ("wq", wqt, nc.sync),
                ("wk", wkt, nc.scalar),
                ("wv", wvt, nc.gpsimd),
                ("wo", wot, nc.sync),
            ):
                w = const.tile([128, 3, C], BF16, tag=name)
                eng.dma_start(out=w, in_=dram.rearrange("(k p) d -> p k d", p=128))
                w_sb[name] = w

            def load_group(g, xt=None):
                """DMA xT for group g, allocate qt/kt/v_aug tiles."""
                if xt is None:
                    xt = xp.tile([128, 3, GB, T], BF16, name=f"xt{g}")
                    for bi in range(GB):
                        nc.gpsimd.dma_start(
                            out=xt[:, :, bi, :],
                            in_=xT[g * GB + bi].rearrange(
                                "(k p) t -> p k t", p=128
                            ),
                        )
                qt = qkv.tile([128, 3, GB, T], BF16, tag="qt", name=f"qt{g}")
                kt = qkv.tile([128, 3, GB, T], BF16, tag="kt", name=f"kt{g}")
                vs = []
                for bi in range(GB):
                    # head stride 68 (not 65) keeps every rhs slice 8B-aligned
                    v = qkv.tile(
                        [128, 2, H, 68], BF16, tag=f"v{bi}", name=f"v{g}_{bi}"
                    )
                    # constant 1.0 column 64 -> AV matmul emits the softmax
                    # denominator for free
                    nc.gpsimd.memset(v[:, :, :, 64:65], 1.0)
                    vs.append(v)
                return xt, qt, kt, vs

            def proj_emitters(xt, qt, kt, vs):
                """Closures each emitting one PSUM-chunk of the QK/V
                projections (3 accumulating matmuls + 1 evacuation). Ordered
                so the consumers' dependencies resolve earliest-first."""
                def qk_em(dst, wname, d):
                    def em():
                        ps = psProj.tile([128, GB * T], FP32, tag="proj",
                                         name="psqk")
                        for k in range(3):
                            nc.tensor.matmul(
                                ps,
                                lhsT=w_sb[wname][:, k, d * 128:(d + 1) * 128],
                                rhs=xt[:, k, :, :],
                                start=(k == 0), stop=(k == 2),
                            )
                        # chunk 1 evacuates on DVE to balance ACT/DVE load
                        if d == 1:
                            nc.vector.tensor_copy(dst[:, d, :, :], ps)
                        else:
                            nc.scalar.copy(dst[:, d, :, :], ps)
                    return em

                def v_em(bi, t2):
                    def em():
                        ps = psProj.tile([128, GB * T], FP32, tag="proj",
                                         name="psv")
                        for k in range(3):
                            nc.tensor.matmul(
                                ps[:, 0:C],
                                lhsT=xt[:, k, bi, t2 * 128:(t2 + 1) * 128],
                                rhs=w_sb["wv"][:, k, :],
                                start=(k == 0), stop=(k == 2),
                            )
                        nc.vector.tensor_copy(
                            vs[bi][:, t2, :, 0:64],
                            ps[:, 0:C].rearrange("p (h j) -> p h j", j=64),
                        )
                    return em

                return [
                    qk_em(qt, "wq", 0), qk_em(kt, "wk", 0),
                    v_em(0, 0), v_em(0, 1),
                    qk_em(qt, "wq", 1), qk_em(kt, "wk", 1),
                    qk_em(qt, "wq", 2), qk_em(kt, "wk", 2),
                    v_em(1, 0), v_em(1, 1),
                ]

            def att_stage1(qt, kt, bi, pair, tail=False):
                """Scores (PE) + exp (ACT) + causal mask (GpSimd) -> masked P_T.

                Block layout per sub: [tri(ts0,tq0) | full(ts0,tq1) |
                tri(ts1,tq1)] so the ts0 row is ONE N=256 matmul; emission
                alternates subs so consecutive matmuls sit in different PE
                row-groups (rows 0-63 vs 64-127) and can overlap."""
                # per-sub score tiles: one PSUM bank each (3 bufs) frees a
                # bank for psProj's third buffer, decoupling projection
                # matmuls from evacuation latency
                scs = [psSc.tile([128, 512], FP32, tag="sc", name=f"sc{s}")
                       for s in range(2)]
                kts = [kt[s * 64:s * 64 + 64, pair, bi, :] for s in range(2)]
                qts = [qt[s * 64:s * 64 + 64, pair, bi, :] for s in range(2)]
                for s in range(2):
                    nc.tensor.matmul(
                        scs[s][:, 0:256], lhsT=kts[s][:, 0:128],
                        rhs=qts[s][:, 0:256], start=True, stop=True,
                    )
                for s in range(2):
                    nc.tensor.matmul(
                        scs[s][:, 256:384], lhsT=kts[s][:, 128:256],
                        rhs=qts[s][:, 128:256], start=True, stop=True,
                    )
                # ---- exp on ACT (one call per sub) ----
                p_t = pp.tile([128, 2, 3, 128], BF16, tag="p")
                for s in range(2):
                    nc.scalar.activation(
                        p_t[:, s, :, :],
                        scs[s][:, 0:384].rearrange("p (k c) -> p k c", c=128),
                        AFT.Exp, scale=SCALE,
                    )
                # ---- causal mask: GpSimd (otherwise idle) in steady state;
                # DVE in the last group, where the chain latency is exposed
                # and DVE has slack ----
                for blk in (0, 2):
                    if tail:
                        nc.vector.tensor_mul(
                            p_t[:, :, blk, :], p_t[:, :, blk, :],
                            mtriT[:, None, :].to_broadcast((128, 2, 128)),
                        )
                    else:
                        nc.gpsimd.affine_select(
                            out=p_t[:, :, blk, :], in_=p_t[:, :, blk, :],
                            compare_op=mybir.AluOpType.is_ge,
                            fill=0.0, base=0,
                            pattern=[[0, 2], [1, 128]],
                            channel_multiplier=-1,
                        )
                return p_t

            def att_stage2(p_t, v, pair, o_sbt):
                """AV matmuls + fused denominator + normalize/evacuate."""
                # block stride 66 fp32 = 264B keeps matmul PSUM outputs
                # 8B-aligned (PSUM cacheline)
                po = psPo.tile([128, 2, 2, 66], FP32, tag="po", name="po")
                for s in range(2):
                    h = 2 * pair + s
                    nc.tensor.matmul(
                        po[:, 0, s, 0:65], lhsT=p_t[:, s, 0, :],
                        rhs=v[:, 0, h, 0:65], start=True, stop=True,
                    )
                    nc.tensor.matmul(
                        po[:, 1, s, 0:65], lhsT=p_t[:, s, 1, :],
                        rhs=v[:, 0, h, 0:65], start=True, stop=False,
                    )
                    nc.tensor.matmul(
                        po[:, 1, s, 0:65], lhsT=p_t[:, s, 2, :],
                        rhs=v[:, 1, h, 0:65], start=False, stop=True,
                    )
                # ---- normalize: per-partition (=per-tq) reciprocal, then
                # one broadcast multiply that doubles as the PSUM->SBUF
                # evacuation ----
                rs = st.tile([128, 2, 2], FP32, tag="rs")
                nc.vector.reciprocal(rs, po[:, :, :, 64])
                out_sl = o_sbt[:, :, pair * 128:(pair + 1) * 128].rearrange(
                    "p t (s j) -> p t s j", j=64
                )
                nc.vector.tensor_mul(
                    out_sl, po[:, :, :, 0:64],
                    rs[:, :, :, None].to_broadcast((128, 2, 2, 64)),
                )

            def emit_trans(o_sbt, last=False):
                otp_t = otp.tile([128, 2, 3, 128], BF16)
                if last:
                    # final batch: transpose on the PE (idle during the
                    # drain) instead of the backed-up sync DMA queue
                    for tqb in range(2):
                        pst = psSc.tile([128, 3, 128], BF16, tag="sc",
                                        name="pstr")
                        for k in range(3):
                            nc.tensor.transpose(
                                pst[:, k, :],
                                o_sbt[:, tqb, k * 128:(k + 1) * 128],
                                ident,
                            )
                        nc.scalar.copy(otp_t[:, tqb, :, :], pst)
                    return otp_t
                for tqb in range(2):
                    nc.sync.dma_start(
                        out=otp_t[:, tqb, :, :],
                        in_=o_sbt[:, tqb, :].rearrange("p (k c) -> p k c", c=128),
                        transpose=True,
                    )
                return otp_t

            def yproj_halves(b, otp_t):
                """Two closures, one per tq-block, so the y-projection can be
                spread across filler slots; the store fires with the second."""
                ys = yp.tile([128, 2, C], BF16)

                def half(tqb):
                    def em():
                        ps = psProj.tile([128, GB * T], FP32, tag="proj",
                                         name="psy")
                        for k in range(3):
                            nc.tensor.matmul(
                                ps[:, 0:C],
                                lhsT=otp_t[:, tqb, k, :],
                                rhs=w_sb["wo"][:, k, :],
                                start=(k == 0), stop=(k == 2),
                            )
                        nc.vector.tensor_copy(ys[:, tqb, :], ps[:, 0:C])
                        if tqb == 1:
                            # y stores on the sync HWDGE queue: keeps the
                            # end-of-kernel SWDGE drain off the critical tail
                            nc.sync.dma_start(
                                out=y[b].rearrange("(t2 p) c -> p t2 c", p=128),
                                in_=ys,
                            )
                    return em

                return [half(0), half(1)]

            # ---- prologue: only the chunks att (b0, p0) needs up front;
            # the rest of group 0's projections become its own filler.
            # y-projections join the filler pool as half-batch units and act
            # as the drought filler once a group's chunks run dry (notably
            # the last group, which has no next-group chunks). ----
            cur = load_group(0, xt=xt0)
            g0_ems = proj_emitters(cur[0], cur[1], cur[2], cur[3])
            for em in g0_ems[:4]:
                em()
            carry = g0_ems[4:]
            pending_y = []
            pending2 = []  # two-deep attention software pipeline (AV+norm
            # of att k runs after scores/exp/mask of att k+2 are emitted)

            def flush_y(n=99):
                while pending_y and n > 0:
                    pending_y.pop(0)()
                    n -= 1

            def run_pending2(keep=0):
                while len(pending2) > keep:
                    pending2.pop(0)()

            for g in range(NG):
                nxt = None
                ems = list(carry)
                carry = []
                if g + 1 < NG:
                    nxt = load_group(g + 1)
                    ems += proj_emitters(nxt[0], nxt[1], nxt[2], nxt[3])
                _, qt, kt, vs = cur
                ei = [0]

                att_budget = [0]

                def filler(ems=ems, ei=ei):
                    # one projection chunk, emitted inside the softmax wait
                    # so the PE always has independent work; falls back to a
                    # pending y-projection half (at most one per att) when
                    # chunks run dry
                    if ei[0] < len(ems):
                        ems[ei[0]]()
                        ei[0] += 1
                    elif att_budget[0] > 0:
                        att_budget[0] -= 1
                        flush_y(1)

                for bi in range(GB):
                    b = g * GB + bi
                    o_sbt = osb.tile([128, 2, C], BF16)
                    for pair in range(3):
                        att_budget[0] = 1
                        filler()
                        tail = g == NG - 1
                        p_t = att_stage1(qt, kt, bi, pair, tail=tail)
                        # the AV+norm of the att two (three in the tail
                        # group) pairs back lands here, giving the PE ready
                        # work while exp/mask run
                        run_pending2(keep=1 if (tail and bi == GB - 1)
                                     else 2)
                        filler()
                        # keep a reserve of deferred y-projection halves so
                        # the last group (which has no next-group projection
                        # chunks) still has PE filler work
                        if pair == 2 and len(pending_y) > 6:
                            flush_y(2)

                        def stage2(p_t=p_t, v=vs[bi], pair=pair,
                                   o_sbt=o_sbt, b=b):
                            att_stage2(p_t, v, pair, o_sbt)
                            if pair == 2:
                                pending_y.extend(
                                    yproj_halves(
                                        b, emit_trans(o_sbt, last=b == BL - 1)
                                    )
                                )
                        pending2.append(stage2)
                while ei[0] < len(ems):
                    ems[ei[0]]()
                    ei[0] += 1
                cur = nxt
            run_pending2(keep=0)
            flush_y()
    return nc


_NC = None


def _get_nc():
    global _NC
    if _NC is None:
        _NC = split_multi_waits(build_kernel())
    return _NC


def kernel(x, Wq, Wk, Wv, Wo, _trace=False):
    bf16 = ml_dtypes.bfloat16
    wq_t = np.ascontiguousarray(Wq.T).astype(bf16)
    wk_t = np.ascontiguousarray(Wk.T).astype(bf16)
    wv_t = np.ascontiguousarray(Wv.T).astype(bf16)
    wo_t = np.ascontiguousarray(Wo.T).astype(bf16)
    in_maps = []
    for i in range(N_CORES):
        xs = x[i * BL : (i + 1) * BL]  # [BL, T, C]
        xs_t = np.ascontiguousarray(xs.transpose(0, 2, 1)).astype(bf16)
        in_maps.append(
            {"xT": xs_t, "wqt": wq_t, "wkt": wk_t, "wvt": wv_t, "wot": wo_t}
        )
    res = run_bass_kernel_spmd(
        _get_nc(), in_maps, list(range(N_CORES)), trace=_trace
    )
    out = np.concatenate([r["y"] for r in res.results], axis=0)
    if _trace:
        return out.astype(np.float32), res
    return out.astype(np.float32)
